# revision 26
# baseline (speedup 1.0000x reference)
"""Causal self-attention (B=4, S=2048, D=1024, single head, fp32) on 8 trn2
NeuronCores.

Sharding: core 2*b + c handles batch b with the parity-c half of the keys
(global key rows 2*i + c), over ALL queries — a flash-attention split over
the key dimension. Each core returns unnormalized softmax numerators
o = sum_k exp(s*scale) v plus per-row l = sum exp; the host combines the
two key-halves exactly (no max subtraction needed: |logits| <~ 6).

SPMD trick: one program serves both parities. The host pair-swaps the rows
of x for odd cores (rows [1,0,3,2,...]), so each core's keys sit at even
row positions and the on-chip stride-2 access pattern is parity-free. The
causal diagonal mask (which depends on the parity) ships as a small
per-core input; the host pair-swaps the outputs of odd cores back.

x ships pre-transposed from the host (layout prep is host-side, like the
pair-swap). Everything runs in fp16 (1 cyc/row on the PE like bf16, but 8x
the mantissa — rel err ~7e-4 vs 2e-2 budget — and FWL weight loads apply,
unlike f32r). All weights are DMA'd once, up front, and stay resident in
SBUF (fp16 makes them fit); x's first half is split across both DMA
queues so the first projection matmul can start ~5us in. Scores are
computed TRANSPOSED ([key, query] tiles) so the exp output feeds attn @ v
directly as the stationary operand — no PE transposes and no PSUM->SBUF
attn copies. The softmax denominator l comes from a ones-vector matmul
accumulated across key blocks. o ships back as fp16 (|o| <~ 1.4e4, well
inside fp16 range; the host combine is float64).
"""
import math
import numpy as np

import concourse.bacc as bacc
import concourse.mybir as mybir
from concourse import tile
from concourse.bass_utils import run_bass_kernel_spmd
from concourse.masks import make_identity

B, S, D = 4, 2048, 1024
P = 128
DT = D // P          # 8 d-tiles (contraction)
ET = D // P          # 8 e-tiles (output feature)
HKT = 8              # compacted key tiles per core (1024 keys)
NMB = 8              # 256-query mega blocks
QB = 256
INV_SQRT_D = 1.0 / math.sqrt(D)
NEG = -1e30

F32 = mybir.dt.float32
F16 = mybir.dt.float16

_CACHED_NC = None


def build_nc():
    nc = bacc.Bacc("TRN2", target_bir_lowering=False)
    # x ships twice, pre-transposed and chunk-major (block h*DT+dt):
    # xk = this core's 1024 compacted keys only (2MB — the A2 critical set
    # is just 1MB of it), xq = all 2048 queries in natural order (4MB,
    # needed only by A4 much later). Separating them shrinks the startup
    # critical set and removes the stride-2 key access pattern.
    xk_p = nc.declare_dram_parameter("xk", [DT * 2 * P, 512], F16,
                                     isOutput=False)
    xq_p = nc.declare_dram_parameter("xq", [DT * 2 * P, S // 2], F16,
                                     isOutput=False)
    # weights ship host-pre-tiled so every DMA run is contiguous per
    # partition row (the naive [D, D] layout gathers 256B runs and is
    # ~10x slower, descriptor-bound)
    wq_p = nc.declare_dram_parameter("wq", [P, ET * DT * P], F16,
                                     isOutput=False)
    wk_p = nc.declare_dram_parameter("wk", [P, ET * DT * P], F16,
                                     isOutput=False)
    wv_p = nc.declare_dram_parameter("wv", [P, 2 * DT * 512], F16,
                                     isOutput=False)
    mask_p = nc.declare_dram_parameter("mask", [P, QB], F16, isOutput=False)
    o_p = nc.declare_dram_parameter("o", [S, D], F16, isOutput=True)
    l_p = nc.declare_dram_parameter("l", [1, S], F32, isOutput=True)
    dbg_p = nc.declare_dram_parameter("dbg", [P, 1], F32, isOutput=True)

    with tile.TileContext(nc) as tc:
        # ---- persistent pools (bottom of SBUF stack) ----
        with (
            tc.tile_pool(name="qT_pool", bufs=1) as qT_pool,
            tc.tile_pool(name="kT_pool", bufs=1) as kT_pool,
            tc.tile_pool(name="v_pool", bufs=1) as v_pool,
            tc.tile_pool(name="const_pool", bufs=1) as const_pool,
        ):
            # four per-ch tiles (not one) so phase B's dependency on a
            # query-column range resolves against just that ch's copies
            qTs = [qT_pool.tile([P, ET, 512], F16, tag=f"qT{ch}",
                                name=f"qT{ch}") for ch in range(4)]
            kT = kT_pool.tile([P, ET, HKT * P], F16)   # [e_p, et, s_k] 16KB/p
            vv = v_pool.tile([P, HKT, D], F16)         # [s_k_p, st, e] 16KB/p
            mask_sb = const_pool.tile([P, QB], F16)
            ident = const_pool.tile([P, P], F16)
            ones_sb = const_pool.tile([P, 1], F16)
            l_sb = const_pool.tile([1, S], F32)
            warm = const_pool.tile([P, 512], F16)
            dbg_sb = const_pool.tile([P, 1], F32)
            nc.vector.memset(warm[:], 0.25)
            make_identity(nc, ident[:])
            nc.gpsimd.memset(ones_sb[:], 1.0)

            # ================= Phase A: x^T + projections =================
            with (
                tc.tile_pool(name="xT_pool", bufs=1) as xT_pool,
                tc.tile_pool(name="w_pool", bufs=1) as w_pool,
                tc.tile_pool(name="psA_all", bufs=1, space="PSUM") as psAll,
            ):
                xT = xT_pool.tile([P, DT, S], F16)     # queries, 32KB/p
                xkT = xT_pool.tile([P, DT, HKT * P], F16)  # keys, 16KB/p
                wk_res = w_pool.tile([P, ET, DT, P], F16)   # 16KB/p
                wq_res = w_pool.tile([P, ET, DT, P], F16)   # 16KB/p
                wv_res = w_pool.tile([P, 2, DT, 512], F16)  # 16KB/p
                psb = [psAll.tile([P, 512], F32, tag=f"b{i}", name=f"psb{i}")
                       for i in range(8)]

                # A1: ALL input DMAs issued up front, as FEW ops as
                # possible: each dma_start costs ~0.75us of descriptor
                # generation on its issuing engine, and the 16 HW DMA
                # engines drain descriptors in issue order at ~300GB/s
                # aggregate (first byte lands only ~4-9us in). Critical set
                # for A2 = x first half + Wk et0, so: x h0 split across the
                # two queues as one 1MB op each, then wk0, then the rest.
                # xt_p rows are h-major: block (h*DT+dt).
                for dt in range(DT):
                    if dt < 4:
                        nc.sync.dma_start(
                            out=wk_res[:, dt],
                            in_=wk_p[:, dt * DT * P:(dt + 1) * DT * P])
                    nc.gpsimd.dma_start(
                        out=xkT[:, dt, 0:512],
                        in_=xk_p[dt * P:(dt + 1) * P, :])
                for dt in range(4, 8):
                    nc.gpsimd.dma_start(
                        out=wk_res[:, dt],
                        in_=wk_p[:, dt * DT * P:(dt + 1) * DT * P])
                for dt in range(DT):
                    nc.gpsimd.dma_start(
                        out=xkT[:, dt, 512:1024],
                        in_=xk_p[(DT + dt) * P:(DT + dt + 1) * P, :])
                for eb in range(2):
                    nc.gpsimd.dma_start(
                        out=wv_res[:, eb],
                        in_=wv_p[:, eb * DT * 512:(eb + 1) * DT * 512])
                nc.sync.dma_start(out=mask_sb[:], in_=mask_p[:])
                for dt in range(DT):
                    nc.sync.dma_start(
                        out=xT[:, dt, 0:1024],
                        in_=xq_p[dt * P:(dt + 1) * P, :])
                for dt in range(DT):
                    nc.gpsimd.dma_start(
                        out=xT[:, dt, 1024:2048],
                        in_=xq_p[(DT + dt) * P:(DT + dt + 1) * P, :])
                for et in range(ET):
                    nc.sync.dma_start(
                        out=wq_res[:, et],
                        in_=wq_p[:, et * DT * P:(et + 1) * DT * P])

                # PE warmup: keeps the tensor engine busy (and its clock
                # ramping) while the first x/wk chunks stream in. The psum
                # result feeds a dummy output so it isn't dead-code.
                for w in range(18):
                    nc.tensor.matmul(psb[7][:], warm[:, :P], warm[:],
                                     start=(w == 0), stop=(w == 17))
                nc.vector.tensor_copy(dbg_sb[:], psb[7][:, :1])

                # A2: kT[e, i] = sum_d Wk[d, e] * x_key[i, d].  ch-outer so
                # the first pass only needs the first half of x.
                wu = [18]

                def _warm_fill():
                    # dependency-free matmul the PE can run while the next
                    # x/w chunk is still in flight (first et pass only)
                    nc.tensor.matmul(psb[6][:], warm[:, :P], warm[:],
                                     start=(wu[0] == 18), stop=False)
                    wu[0] += 1

                for ch in range(2):
                    for et in range(ET):
                        ps = psb[ch * 4 + (et % 4)]
                        for d in range(DT):
                            if ch == 0 and et == 0:
                                _warm_fill()
                            nc.tensor.matmul(
                                ps[:],
                                wk_res[:, et, d],
                                xkT[:, d, ch * 512:(ch + 1) * 512],
                                start=(d == 0), stop=(d == DT - 1))
                        nc.vector.tensor_copy(
                            kT[:, et, ch * 512:(ch + 1) * 512], ps[:])
                nc.tensor.matmul(psb[6][:], warm[:, :P], warm[:],
                                 start=False, stop=True)
                nc.vector.tensor_copy(dbg_sb[:], psb[6][:, :1])
                nc.sync.dma_start(out=dbg_p[:], in_=dbg_sb[:])

                # A3: v[i, e] = sum_d x_key[i, d] * Wv[d, e]   (8 psum banks)
                for eb in range(2):
                    for d in range(DT):
                        for st in range(HKT):
                            nc.tensor.matmul(
                                psb[st][:],
                                xkT[:, d, st * P:(st + 1) * P],
                                wv_res[:, eb, d],
                                start=(d == 0), stop=(d == DT - 1))
                    for st in range(HKT):
                        nc.vector.tensor_copy(
                            vv[:, st, eb * 512:(eb + 1) * 512], psb[st][:])

                # A4: qT[e, s] = sum_d Wq[d, e] * x[s, d]  (all queries)
                # ch-outer: phase B's m-th mega block needs qT columns
                # m*256.., so finish low query columns across all et first.
                for ch in range(4):
                    for et in range(ET):
                        ps = psb[et]
                        for d in range(DT):
                            nc.tensor.matmul(
                                ps[:],
                                wq_res[:, et, d],
                                xT[:, d, ch * 512:(ch + 1) * 512],
                                start=(d == 0), stop=(d == DT - 1))
                        nc.vector.tensor_copy(qTs[ch][:, et], ps[:])

            # ===== Phase B: causal attention, scores kept transposed =====
            # psS[k, q] tiles: exp output is already [key, query] so it is
            # the stationary operand of attn @ v directly — no transposes.
            with (
                tc.tile_pool(name="at_pool", bufs=16) as at_pool,
                tc.tile_pool(name="ob_pool", bufs=2) as ob_pool,
                tc.tile_pool(name="psS_pool", bufs=3, space="PSUM") as psS_pool,
                tc.tile_pool(name="psL_pool", bufs=1, space="PSUM") as psL_pool,  # 1 bank, ping-pong halves
                tc.tile_pool(name="psO_pool", bufs=2, space="PSUM") as psO_pool,
            ):
                psLb = psL_pool.tile([1, 2, QB], F32, tag="psL",
                                     name="psLbank")
                for m in range(NMB):
                    nkb = m + 1          # valid compacted key blocks
                    attn_ts = {}
                    psL = psLb[:, m % 2]
                    qm = qTs[m // 2][:, :, (m % 2) * QB:(m % 2) * QB + QB]

                    def _scores(kb, diag):
                        psS = psS_pool.tile([P, QB], F32, tag="psS",
                                            name=f"psS{m}_{kb}")
                        for et in range(ET):
                            nc.tensor.matmul(
                                psS[:],
                                kT[:, et, kb * P:(kb + 1) * P],
                                qm[:, et],
                                start=(et == 0),
                                stop=(et == ET - 1 and not diag))
                        if diag:
                            # causal mask folded into the PE accumulation:
                            # ident^T @ mask == mask — no DVE op in the
                            # critical chain
                            nc.tensor.matmul(psS[:], ident[:], mask_sb[:],
                                             start=False, stop=True)
                        at = at_pool.tile([P, QB], F16, tag="attn",
                                          name=f"attn{m}_{kb}")
                        nc.scalar.activation(
                            at[:], psS[:],
                            mybir.ActivationFunctionType.Exp,
                            scale=INV_SQRT_D)
                        attn_ts[kb] = at
                        # l partial: column sums via ones^T @ attn
                        nc.tensor.matmul(psL[:], ones_sb[:], at[:],
                                         start=(kb == 0), stop=diag)

                    for kb in range(m):
                        _scores(kb, False)
                    # diagonal scores issue now; the off-diagonal attn@v
                    # matmuls below keep the PE busy while its exp runs
                    _scores(m, True)

                    # o = attn^T @ v per 128-query sub-block
                    for half in range(2):
                        jj = 2 * m + half
                        nkb_j = jj // 2 + 1
                        psO = [psO_pool.tile([P, 512], F32, tag=f"psO{eb}",
                                             name=f"psO{jj}_{eb}")
                               for eb in range(2)]
                        for eb in range(2):
                            for kb in range(nkb_j):
                                nc.tensor.matmul(
                                    psO[eb][:],
                                    attn_ts[kb][:, half * P:(half + 1) * P],
                                    vv[:, kb, eb * 512:(eb + 1) * 512],
                                    start=(kb == 0), stop=(kb == nkb_j - 1))
                            o_sb = ob_pool.tile([P, 512], F16, tag=f"o{eb}",
                                                name=f"o{jj}_{eb}")
                            if m == NMB - 1 and eb == 0:
                                # tail: run the two copies on different
                                # engines so they drain in parallel
                                nc.scalar.copy(o_sb[:], psO[eb][:])
                            else:
                                nc.vector.tensor_copy(o_sb[:], psO[eb][:])
                            nc.sync.dma_start(
                                out=o_p[jj * P:(jj + 1) * P,
                                        eb * 512:(eb + 1) * 512],
                                in_=o_sb[:])
                    nc.vector.tensor_copy(
                        l_sb[:, m * QB:(m + 1) * QB], psL[:])
                    nc.gpsimd.dma_start(
                        out=l_p[:, m * QB:(m + 1) * QB],
                        in_=l_sb[:, m * QB:(m + 1) * QB])
    nc.finalize()
    return nc


def _diag_mask(c):
    """mask[i, qq]: 0 if compacted key i of the diagonal block is causally
    valid for query column qq of a 256-query mega block, else -1e30.

    Key i is global row 256*m + 2*i + c; the query at column qq is global
    row 256*m + qq. Valid iff 2*i + c <= qq; m-independent.
    """
    mask = np.full((P, QB), -1e4, dtype=np.float16)
    for qq in range(QB):
        lim = (qq - c) // 2
        if lim >= 0:
            mask[:min(lim + 1, P), qq] = 0.0
    return mask


def _tile_wqk(W):
    # [P, (et*DT+dt)*P + e] = W[dt*P+p, et*P+e]
    W = np.asarray(W, dtype=np.float16)
    return np.ascontiguousarray(
        W.reshape(DT, P, ET, P).transpose(1, 2, 0, 3).reshape(P, -1))


def _tile_wv(W):
    # [P, (eb*DT+d)*512 + e] = W[d*P+p, eb*512+e]
    W = np.asarray(W, dtype=np.float16)
    return np.ascontiguousarray(
        W.reshape(DT, P, 2, 512).transpose(1, 2, 0, 3).reshape(P, -1))


def _make_in_maps(x, Wq, Wk, Wv):
    x = np.asarray(x, dtype=np.float16)
    Wq = _tile_wqk(Wq)
    Wk = _tile_wqk(Wk)
    Wv = _tile_wv(Wv)
    masks = [_diag_mask(0), _diag_mask(1)]
    in_maps = []
    for core in range(8):
        b, c = core // 2, core % 2
        xq = x[b].T.reshape(DT, P, 2, S // 2).transpose(2, 0, 1, 3)
        xk = np.ascontiguousarray(x[b][c::2]).T.reshape(
            DT, P, 2, 512).transpose(2, 0, 1, 3)
        in_maps.append({
            "xk": np.ascontiguousarray(xk.reshape(DT * 2 * P, 512)),
            "xq": np.ascontiguousarray(xq.reshape(DT * 2 * P, S // 2)),
            "wq": Wq, "wk": Wk, "wv": Wv,
            "mask": masks[c],
        })
    return in_maps


def _combine(res):
    out = np.empty((B, S, D), dtype=np.float32)
    for b in range(B):
        r0, r1 = res.results[2 * b], res.results[2 * b + 1]

        def stat(r, key):
            return np.ascontiguousarray(r[key]).reshape(S, 1)
        o0 = r0["o"].astype(np.float64)
        l0 = stat(r0, "l").astype(np.float64)
        o1 = r1["o"].astype(np.float64)
        l1 = stat(r1, "l").astype(np.float64)
        out[b] = ((o0 + o1) / (l0 + l1)).astype(np.float32)
    return out


def kernel(x, Wq, Wk, Wv):
    global _CACHED_NC
    if _CACHED_NC is None:
        _CACHED_NC = build_nc()
    in_maps = _make_in_maps(x, Wq, Wk, Wv)
    res = run_bass_kernel_spmd(_CACHED_NC, in_maps, list(range(8)))
    return _combine(res)


# revision 27
# speedup vs baseline: 1.0045x; 1.0045x over previous
"""Causal self-attention (B=4, S=2048, D=1024, single head, fp32) on 8 trn2
NeuronCores.

Sharding: core 2*b + c handles batch b with the parity-c half of the keys
(global key rows 2*i + c), over ALL queries — a flash-attention split over
the key dimension. Each core returns unnormalized softmax numerators
o = sum_k exp(s*scale) v plus per-row l = sum exp; the host combines the
two key-halves exactly (no max subtraction needed: |logits| <~ 6).

SPMD trick: one program serves both parities. The host pair-swaps the rows
of x for odd cores (rows [1,0,3,2,...]), so each core's keys sit at even
row positions and the on-chip stride-2 access pattern is parity-free. The
causal diagonal mask (which depends on the parity) ships as a small
per-core input; the host pair-swaps the outputs of odd cores back.

x ships pre-transposed from the host (layout prep is host-side, like the
pair-swap). Everything runs in fp16 (1 cyc/row on the PE like bf16, but 8x
the mantissa — rel err ~7e-4 vs 2e-2 budget — and FWL weight loads apply,
unlike f32r). All weights are DMA'd once, up front, and stay resident in
SBUF (fp16 makes them fit); x's first half is split across both DMA
queues so the first projection matmul can start ~5us in. Scores are
computed TRANSPOSED ([key, query] tiles) so the exp output feeds attn @ v
directly as the stationary operand — no PE transposes and no PSUM->SBUF
attn copies. The softmax denominator l comes from a ones-vector matmul
accumulated across key blocks. o ships back as fp16 (|o| <~ 1.4e4, well
inside fp16 range; the host combine is float64).
"""
import math
import numpy as np

import concourse.bacc as bacc
import concourse.mybir as mybir
from concourse import tile
from concourse.bass_utils import run_bass_kernel_spmd
from concourse.masks import make_identity

B, S, D = 4, 2048, 1024
P = 128
DT = D // P          # 8 d-tiles (contraction)
ET = D // P          # 8 e-tiles (output feature)
HKT = 8              # compacted key tiles per core (1024 keys)
NMB = 8              # 256-query mega blocks
QB = 256
INV_SQRT_D = 1.0 / math.sqrt(D)
NEG = -1e30

F32 = mybir.dt.float32
F16 = mybir.dt.float16

_CACHED_NC = None


def build_nc():
    nc = bacc.Bacc("TRN2", target_bir_lowering=False)
    # x ships twice, pre-transposed and chunk-major (block h*DT+dt):
    # xk = this core's 1024 compacted keys only (2MB — the A2 critical set
    # is just 1MB of it), xq = all 2048 queries in natural order (4MB,
    # needed only by A4 much later). Separating them shrinks the startup
    # critical set and removes the stride-2 key access pattern.
    xk_p = nc.declare_dram_parameter("xk", [DT * 2 * P, 512], F16,
                                     isOutput=False)
    xq_p = nc.declare_dram_parameter("xq", [DT * 2 * P, S // 2], F16,
                                     isOutput=False)
    # weights ship host-pre-tiled so every DMA run is contiguous per
    # partition row (the naive [D, D] layout gathers 256B runs and is
    # ~10x slower, descriptor-bound)
    wq_p = nc.declare_dram_parameter("wq", [P, ET * DT * P], F16,
                                     isOutput=False)
    wk_p = nc.declare_dram_parameter("wk", [P, ET * DT * P], F16,
                                     isOutput=False)
    wv_p = nc.declare_dram_parameter("wv", [P, 2 * DT * 512], F16,
                                     isOutput=False)
    mask_p = nc.declare_dram_parameter("mask", [P, QB], F16, isOutput=False)
    o_p = nc.declare_dram_parameter("o", [S, D], F16, isOutput=True)
    l_p = nc.declare_dram_parameter("l", [1, S], F32, isOutput=True)
    dbg_p = nc.declare_dram_parameter("dbg", [P, 1], F32, isOutput=True)

    with tile.TileContext(nc) as tc:
        # ---- persistent pools (bottom of SBUF stack) ----
        with (
            tc.tile_pool(name="qT_pool", bufs=1) as qT_pool,
            tc.tile_pool(name="kT_pool", bufs=1) as kT_pool,
            tc.tile_pool(name="v_pool", bufs=1) as v_pool,
            tc.tile_pool(name="const_pool", bufs=1) as const_pool,
        ):
            # four per-ch tiles (not one) so phase B's dependency on a
            # query-column range resolves against just that ch's copies
            qTs = [qT_pool.tile([P, ET, 512], F16, tag=f"qT{ch}",
                                name=f"qT{ch}") for ch in range(4)]
            kT = kT_pool.tile([P, ET, HKT * P], F16)   # [e_p, et, s_k] 16KB/p
            vv = v_pool.tile([P, HKT, D], F16)         # [s_k_p, st, e] 16KB/p
            mask_sb = const_pool.tile([P, QB], F16)
            ident = const_pool.tile([P, P], F16)
            ones_sb = const_pool.tile([P, 1], F16)
            l_sb = const_pool.tile([1, S], F32)
            warm = const_pool.tile([P, 512], F16)
            dbg_sb = const_pool.tile([P, 1], F32)
            nc.vector.memset(warm[:], 0.25)
            make_identity(nc, ident[:])
            nc.gpsimd.memset(ones_sb[:], 1.0)

            # ================= Phase A: x^T + projections =================
            with (
                tc.tile_pool(name="xT_pool", bufs=1) as xT_pool,
                tc.tile_pool(name="w_pool", bufs=1) as w_pool,
                tc.tile_pool(name="psA_all", bufs=1, space="PSUM") as psAll,
            ):
                xT = xT_pool.tile([P, DT, S], F16)     # queries, 32KB/p
                xkT = xT_pool.tile([P, DT, HKT * P], F16)  # keys, 16KB/p
                wk_res = w_pool.tile([P, ET, DT, P], F16)   # 16KB/p
                wq_res = w_pool.tile([P, ET, DT, P], F16)   # 16KB/p
                wv_res = w_pool.tile([P, 2, DT, 512], F16)  # 16KB/p
                psb = [psAll.tile([P, 512], F32, tag=f"b{i}", name=f"psb{i}")
                       for i in range(8)]

                # A1: ALL input DMAs issued up front, as FEW ops as
                # possible: each dma_start costs ~0.75us of descriptor
                # generation on its issuing engine, and the 16 HW DMA
                # engines drain descriptors in issue order at ~300GB/s
                # aggregate (first byte lands only ~4-9us in). Critical set
                # for A2 = x first half + Wk et0, so: x h0 split across the
                # two queues as one 1MB op each, then wk0, then the rest.
                # xt_p rows are h-major: block (h*DT+dt).
                for dt in range(DT):
                    if dt < 4:
                        nc.sync.dma_start(
                            out=wk_res[:, dt],
                            in_=wk_p[:, dt * DT * P:(dt + 1) * DT * P])
                    nc.gpsimd.dma_start(
                        out=xkT[:, dt, 0:512],
                        in_=xk_p[dt * P:(dt + 1) * P, :])
                for dt in range(4, 8):
                    nc.gpsimd.dma_start(
                        out=wk_res[:, dt],
                        in_=wk_p[:, dt * DT * P:(dt + 1) * DT * P])
                for dt in range(DT):
                    nc.gpsimd.dma_start(
                        out=xkT[:, dt, 512:1024],
                        in_=xk_p[(DT + dt) * P:(DT + dt + 1) * P, :])
                for eb in range(2):
                    nc.gpsimd.dma_start(
                        out=wv_res[:, eb],
                        in_=wv_p[:, eb * DT * 512:(eb + 1) * DT * 512])
                nc.sync.dma_start(out=mask_sb[:], in_=mask_p[:])
                for dt in range(DT):
                    nc.sync.dma_start(
                        out=xT[:, dt, 0:1024],
                        in_=xq_p[dt * P:(dt + 1) * P, :])
                for dt in range(DT):
                    nc.gpsimd.dma_start(
                        out=xT[:, dt, 1024:2048],
                        in_=xq_p[(DT + dt) * P:(DT + dt + 1) * P, :])
                for et in range(ET):
                    nc.sync.dma_start(
                        out=wq_res[:, et],
                        in_=wq_p[:, et * DT * P:(et + 1) * DT * P])

                # PE warmup: keeps the tensor engine busy (and its clock
                # ramping) while the first x/wk chunks stream in. The psum
                # result feeds a dummy output so it isn't dead-code.
                for w in range(18):
                    nc.tensor.matmul(psb[7][:], warm[:, :P], warm[:],
                                     start=(w == 0), stop=(w == 17))
                nc.vector.tensor_copy(dbg_sb[:], psb[7][:, :1])

                # A2: kT[e, i] = sum_d Wk[d, e] * x_key[i, d].  ch-outer so
                # the first pass only needs the first half of x.
                wu = [18]

                def _warm_fill():
                    # dependency-free matmul the PE can run while the next
                    # x/w chunk is still in flight (first et pass only)
                    nc.tensor.matmul(psb[6][:], warm[:, :P], warm[:],
                                     start=(wu[0] == 18), stop=False)
                    wu[0] += 1

                for ch in range(2):
                    for et in range(ET):
                        ps = psb[ch * 4 + (et % 4)]
                        for d in range(DT):
                            if ch == 0 and et == 0:
                                _warm_fill()
                            nc.tensor.matmul(
                                ps[:],
                                wk_res[:, et, d],
                                xkT[:, d, ch * 512:(ch + 1) * 512],
                                start=(d == 0), stop=(d == DT - 1))
                        nc.vector.tensor_copy(
                            kT[:, et, ch * 512:(ch + 1) * 512], ps[:])
                nc.tensor.matmul(psb[6][:], warm[:, :P], warm[:],
                                 start=False, stop=True)
                nc.vector.tensor_copy(dbg_sb[:], psb[6][:, :1])
                nc.sync.dma_start(out=dbg_p[:], in_=dbg_sb[:])

                # A3: v[i, e] = sum_d x_key[i, d] * Wv[d, e]   (8 psum banks)
                for eb in range(2):
                    for d in range(DT):
                        for st in range(HKT):
                            nc.tensor.matmul(
                                psb[st][:],
                                xkT[:, d, st * P:(st + 1) * P],
                                wv_res[:, eb, d],
                                start=(d == 0), stop=(d == DT - 1))
                    for st in range(HKT):
                        nc.vector.tensor_copy(
                            vv[:, st, eb * 512:(eb + 1) * 512], psb[st][:])

                # A4: qT[e, s] = sum_d Wq[d, e] * x[s, d]  (all queries)
                # ch-outer: phase B's m-th mega block needs qT columns
                # m*256.., so finish low query columns across all et first.
                for ch in range(4):
                    for et in range(ET):
                        ps = psb[et]
                        for d in range(DT):
                            nc.tensor.matmul(
                                ps[:],
                                wq_res[:, et, d],
                                xT[:, d, ch * 512:(ch + 1) * 512],
                                start=(d == 0), stop=(d == DT - 1))
                        nc.vector.tensor_copy(qTs[ch][:, et], ps[:])

            # ===== Phase B: causal attention, scores kept transposed =====
            # psS[k, q] tiles: exp output is already [key, query] so it is
            # the stationary operand of attn @ v directly — no transposes.
            with (
                tc.tile_pool(name="at_pool", bufs=12) as at_pool,
                tc.tile_pool(name="ob_pool", bufs=2) as ob_pool,
                tc.tile_pool(name="psS_pool", bufs=3, space="PSUM") as psS_pool,
                tc.tile_pool(name="psL_pool", bufs=1, space="PSUM") as psL_pool,  # 1 bank, ping-pong halves
                tc.tile_pool(name="psO_pool", bufs=2, space="PSUM") as psO_pool,
            ):
                psLb = psL_pool.tile([1, 2, QB], F32, tag="psL",
                                     name="psLbank")
                for m in range(NMB):
                    nkb = m + 1          # valid compacted key blocks
                    attn_ts = {}
                    psL = psLb[:, m % 2]
                    qm = qTs[m // 2][:, :, (m % 2) * QB:(m % 2) * QB + QB]

                    def _scores(kb, diag):
                        psS = psS_pool.tile([P, QB], F32, tag="psS",
                                            name=f"psS{m}_{kb}")
                        for et in range(ET):
                            nc.tensor.matmul(
                                psS[:],
                                kT[:, et, kb * P:(kb + 1) * P],
                                qm[:, et],
                                start=(et == 0),
                                stop=(et == ET - 1 and not diag))
                        if diag:
                            # causal mask folded into the PE accumulation:
                            # ident^T @ mask == mask — no DVE op in the
                            # critical chain
                            nc.tensor.matmul(psS[:], ident[:], mask_sb[:],
                                             start=False, stop=True)
                        at = at_pool.tile([P, QB], F16, tag="attn",
                                          name=f"attn{m}_{kb}")
                        nc.scalar.activation(
                            at[:], psS[:],
                            mybir.ActivationFunctionType.Exp,
                            scale=INV_SQRT_D)
                        attn_ts[kb] = at
                        # l partial: column sums via ones^T @ attn
                        nc.tensor.matmul(psL[:], ones_sb[:], at[:],
                                         start=(kb == 0), stop=diag)

                    for kb in range(m):
                        _scores(kb, False)
                    # diagonal scores issue now; the off-diagonal attn@v
                    # matmuls below keep the PE busy while its exp runs
                    _scores(m, True)

                    # o = attn^T @ v per 128-query sub-block
                    for half in range(2):
                        jj = 2 * m + half
                        nkb_j = jj // 2 + 1
                        psO = [psO_pool.tile([P, 512], F32, tag=f"psO{eb}",
                                             name=f"psO{jj}_{eb}")
                               for eb in range(2)]
                        for eb in range(2):
                            for kb in range(nkb_j):
                                nc.tensor.matmul(
                                    psO[eb][:],
                                    attn_ts[kb][:, half * P:(half + 1) * P],
                                    vv[:, kb, eb * 512:(eb + 1) * 512],
                                    start=(kb == 0), stop=(kb == nkb_j - 1))
                            o_sb = ob_pool.tile([P, 512], F16, tag=f"o{eb}",
                                                name=f"o{jj}_{eb}")
                            if m == NMB - 1 and eb == 0:
                                # tail: run the two copies on different
                                # engines so they drain in parallel
                                nc.scalar.copy(o_sb[:], psO[eb][:])
                            else:
                                nc.vector.tensor_copy(o_sb[:], psO[eb][:])
                            nc.sync.dma_start(
                                out=o_p[jj * P:(jj + 1) * P,
                                        eb * 512:(eb + 1) * 512],
                                in_=o_sb[:])
                    nc.vector.tensor_copy(
                        l_sb[:, m * QB:(m + 1) * QB], psL[:])
                    nc.gpsimd.dma_start(
                        out=l_p[:, m * QB:(m + 1) * QB],
                        in_=l_sb[:, m * QB:(m + 1) * QB])
    nc.finalize()
    return nc


def _diag_mask(c):
    """mask[i, qq]: 0 if compacted key i of the diagonal block is causally
    valid for query column qq of a 256-query mega block, else -1e30.

    Key i is global row 256*m + 2*i + c; the query at column qq is global
    row 256*m + qq. Valid iff 2*i + c <= qq; m-independent.
    """
    mask = np.full((P, QB), -1e4, dtype=np.float16)
    for qq in range(QB):
        lim = (qq - c) // 2
        if lim >= 0:
            mask[:min(lim + 1, P), qq] = 0.0
    return mask


def _tile_wqk(W):
    # [P, (et*DT+dt)*P + e] = W[dt*P+p, et*P+e]
    W = np.asarray(W, dtype=np.float16)
    return np.ascontiguousarray(
        W.reshape(DT, P, ET, P).transpose(1, 2, 0, 3).reshape(P, -1))


def _tile_wv(W):
    # [P, (eb*DT+d)*512 + e] = W[d*P+p, eb*512+e]
    W = np.asarray(W, dtype=np.float16)
    return np.ascontiguousarray(
        W.reshape(DT, P, 2, 512).transpose(1, 2, 0, 3).reshape(P, -1))


def _make_in_maps(x, Wq, Wk, Wv):
    x = np.asarray(x, dtype=np.float16)
    Wq = _tile_wqk(Wq)
    Wk = _tile_wqk(Wk)
    Wv = _tile_wv(Wv)
    masks = [_diag_mask(0), _diag_mask(1)]
    in_maps = []
    for core in range(8):
        b, c = core // 2, core % 2
        xq = x[b].T.reshape(DT, P, 2, S // 2).transpose(2, 0, 1, 3)
        xk = np.ascontiguousarray(x[b][c::2]).T.reshape(
            DT, P, 2, 512).transpose(2, 0, 1, 3)
        in_maps.append({
            "xk": np.ascontiguousarray(xk.reshape(DT * 2 * P, 512)),
            "xq": np.ascontiguousarray(xq.reshape(DT * 2 * P, S // 2)),
            "wq": Wq, "wk": Wk, "wv": Wv,
            "mask": masks[c],
        })
    return in_maps


def _combine(res):
    out = np.empty((B, S, D), dtype=np.float32)
    for b in range(B):
        r0, r1 = res.results[2 * b], res.results[2 * b + 1]

        def stat(r, key):
            return np.ascontiguousarray(r[key]).reshape(S, 1)
        o0 = r0["o"].astype(np.float64)
        l0 = stat(r0, "l").astype(np.float64)
        o1 = r1["o"].astype(np.float64)
        l1 = stat(r1, "l").astype(np.float64)
        out[b] = ((o0 + o1) / (l0 + l1)).astype(np.float32)
    return out


def kernel(x, Wq, Wk, Wv):
    global _CACHED_NC
    if _CACHED_NC is None:
        _CACHED_NC = build_nc()
    in_maps = _make_in_maps(x, Wq, Wk, Wv)
    res = run_bass_kernel_spmd(_CACHED_NC, in_maps, list(range(8)))
    return _combine(res)
